# revision 1
# baseline (speedup 1.0000x reference)
"""AGCNConv (GNN message passing) distributed Bass kernel for 8 TRN2 NeuronCores.

Reference math:
    h   = x @ W
    aew = edge_weight * sigmoid(adaptive_weight)
    deg = segment_sum(aew, row);  dis = where(deg>0, deg^-1/2, 0)
    out = segment_sum(h[col] * (dis[row]*aew*dis[col])[:,None], row)
    out = LeakyReLU(LayerNorm(out + bias))

Key identity: the global factor s = sigmoid(adaptive_weight) cancels in the
symmetric normalization (dis ~ s^-1/2, norm ~ s), so adaptive_weight is unused.

Sharding: nodes (x rows / dest rows) are sharded 8 ways (5000 -> 5120 padded
rows per core); edges are routed to their destination's core and grouped by
(source window, dest block of 128), padded to 128-edge tiles with a schedule
shared across all cores (SPMD: one graph, per-core data).

Device pipeline per core:
  1. h = x @ W (PE transpose + matmul), kept in SBUF.
  2. deg via one-hot matmuls: S[e,d] = (iota==dest_off[e])*ew[e] built in one
     DVE tensor_scalar op per 128-edge tile (bf16); deg_blk += S^T @ 1.
  3. After each half of the dest blocks: dis = masked rsqrt(deg),
     h' = dis*h (bf16), replicate 8x, and AllToAll the half-shard
     (emulates the broken AllGather; two half exchanges let window-A
     gathers start while the second exchange is still in flight).
  4. Edge pass: dma_gather h'[col] rows from HBM in 8192-row chunks over
     4 SWDGE queues (int16 indices => two 20480-row windows = first|second
     halves of all shards), rebuild S per tile, PSUM-accumulate
     out_blk += S^T @ G.  The gather descriptor pipeline (~11 ns/row/core)
     is the measured bottleneck (~0.9 ms of the ~1.1 ms total).
  5. Fused epilogue per block: +w0 partial, *dis[d], +bias, LayerNorm
     (ACT accum_out for mean/var), gamma/beta, LeakyReLU via
     scalar_tensor_tensor max(x, 0.2x), DMA out.
"""

import sys

if "/opt/trn_rl_repo" not in sys.path:
    sys.path.insert(0, "/opt/trn_rl_repo")

import numpy as np

from concourse import bacc, tile, mybir
from concourse.bass_utils import run_bass_kernel_spmd

# ---- problem constants (hardcoded per the harness contract) ----
N = 40000
E = 640000
D = 128
C = 8              # cores
NPC = 5000         # nodes per core
NB = 40            # dest blocks of 128 per core
NPCP = NB * 128    # 5120 padded nodes per core
NFULL = C * NPCP   # 40960
HALF = NPCP // 2   # 2560: half-shard rows; windows = (first|second) halves
WROWS = C * HALF   # 20480 rows per gather window (int16 index limit)
LN_EPS = 1e-5
LEAKY_ALPHA = 0.2

# ---- tunables ----
SKIP = frozenset()   # ablation flags for perf bisection: p1, sa, x, sb, mm, g, epi
CH = 64            # gather chunk size in 128-edge tiles (8192 rows per call)
NQ = 4             # SWDGE queues for gather descriptor parallelism
GDT_BF16 = True    # gather/propagate matmul in bf16 (f32 accumulate in PSUM)

_f32 = mybir.dt.float32
_bf16 = mybir.dt.bfloat16
_i16 = mybir.dt.int16


def _preprocess(edge_index, edge_weight):
    """Route edges to destination cores; group by (window, dest block); build
    the shared padded schedule and per-core tile-layout arrays."""
    row = np.asarray(edge_index[0], dtype=np.int64)
    col = np.asarray(edge_index[1], dtype=np.int64)
    ew = np.asarray(edge_weight, dtype=np.float32)

    core = row // NPC
    lid = row - core * NPC
    blk = lid >> 7
    off = (lid & 127).astype(np.float32)
    s = col // NPC
    r = col - s * NPC                        # 0..4999 within shard
    win = (r >= HALF).astype(np.int64)       # window A: first half of every
    crel = (s * HALF + (r - win * HALF)).astype(np.int16)  # shard; B: second

    # group key per edge: (core, window, block)
    key = (core * 2 + win) * NB + blk
    counts = np.bincount(key, minlength=C * 2 * NB).reshape(C, 2, NB)
    # shared schedule: tiles per (window, block) = max over cores, rounded up
    P = 128 * np.ceil(counts.max(axis=0) / 128.0).astype(np.int64)  # [2, NB]
    for b in range(NB):
        if P[0, b] + P[1, b] == 0:
            P[1, b] = 128  # guarantee at least one (all-dummy) tile per block

    Ppad = int(P.sum())
    T = Ppad // 128
    gstart = np.concatenate([[0], np.cumsum(P.reshape(-1))])[:-1].reshape(2, NB)

    colr_a = np.zeros((C, Ppad), np.int16)
    ew_a = np.zeros((C, Ppad), np.float32)
    dof_a = np.zeros((C, Ppad), np.float32)

    order = np.argsort(key, kind="stable")
    key_s = key[order]
    # position within each (c,w,b) group
    grp_sizes = counts.reshape(-1)
    grp_off = np.concatenate([[0], np.cumsum(grp_sizes)])[:-1]
    within = np.arange(len(key_s)) - grp_off[key_s]
    c_s = key_s // (2 * NB)
    wb_s = key_s % (2 * NB)
    dest = gstart.reshape(-1)[wb_s] + within
    colr_a[c_s, dest] = crel[order]
    ew_a[c_s, dest] = ew[order]
    dof_a[c_s, dest] = off[order]

    # tile-major layouts
    ew_t = np.ascontiguousarray(ew_a.reshape(C, T, 128).transpose(0, 2, 1))      # [C,128,T]
    dof_t = np.ascontiguousarray(dof_a.reshape(C, T, 128).transpose(0, 2, 1))    # [C,128,T]
    # dma_gather index layout: idx i of a tile at [i%16, i//16], replicated x8
    A = colr_a.reshape(C, T, 8, 16).transpose(0, 1, 3, 2)                        # [C,T,16,8]
    idx16 = np.tile(A, (1, 1, 8, 1)).transpose(0, 2, 1, 3).reshape(C, 128, 8 * T)
    idx16 = np.ascontiguousarray(idx16)

    return P, T, ew_t, dof_t, idx16


def _schedule(P):
    """Static tile schedule shared by all cores.

    Returns:
      blocks_a: iteration order for the deg pass: per block, its (w0,w1) tile
                index ranges (contiguous PSUM accumulation groups).
      passes_b: per window, list of (tile_start, ntiles) gather chunks and the
                per-tile block id + run first/last flags.
    """
    P = np.asarray(P)
    tiles = []  # (global tile idx) -> (w, b)
    for w in (0, 1):
        for b in range(NB):
            for _ in range(int(P[w, b]) // 128):
                tiles.append((w, b))
    T = len(tiles)
    t0w = [0, int(P[0].sum()) // 128]
    Tw = [t0w[1], T - t0w[1]]
    return tiles, t0w, Tw


def _build(P, T, tiles, t0w, Tw, sim_single_core=False, reps=1):
    gdt = _bf16 if GDT_BF16 else _f32
    # adaptive SBUF budget: resident tables grow with T (skewed graphs)
    idx_kb = (16 * T) / 1024.0 if T <= 2560 else 2.0
    fixed_kb = 76 + idx_kb + (8 * T) / 1024.0  # h/hp/acc/consts + tables
    ch, gbufs = CH, 3
    while fixed_kb + ch * 0.25 * gbufs + 20 > 155 and (ch > 4 or gbufs > 2):
        if gbufs > 2:
            gbufs -= 1
        else:
            ch //= 2
    nc = bacc.Bacc("TRN2", target_bir_lowering=False, debug=False,
                   enable_asserts=True,
                   num_devices=1 if sim_single_core else C,
                   num_swdge_queues=NQ, dynamic_dma_scratch_size=65536)

    x_in = nc.dram_tensor("x", [NPCP, D], _f32, kind="ExternalInput").ap()
    w_in = nc.dram_tensor("w", [D, D], _f32, kind="ExternalInput").ap()
    bias_in = nc.dram_tensor("bias", [1, D], _f32, kind="ExternalInput").ap()
    gamma_in = nc.dram_tensor("gamma", [1, D], _f32, kind="ExternalInput").ap()
    beta_in = nc.dram_tensor("beta", [1, D], _f32, kind="ExternalInput").ap()
    ewt_in = nc.dram_tensor("ewt", [128, T], _f32, kind="ExternalInput").ap()
    doft_in = nc.dram_tensor("doft", [128, T], _f32, kind="ExternalInput").ap()
    idx_in = nc.dram_tensor("idx16", [128, 8 * T], _i16, kind="ExternalInput").ap()
    out_d = nc.dram_tensor("out", [NPCP, D], _f32, kind="ExternalOutput").ap()

    eq = mybir.AluOpType.is_equal
    mul = mybir.AluOpType.mult
    add = mybir.AluOpType.add
    AF = mybir.ActivationFunctionType

    # per-block window runs: (first_tile, last_tile) or None
    runs = [[None, None] for _ in range(NB)]
    for t, (w, b) in enumerate(tiles):
        if runs[b][w] is None:
            runs[b][w] = [t, t]
        else:
            runs[b][w][1] = t

    with tile.TileContext(nc) as tc:
        with (
            tc.tile_pool(name="const", bufs=1) as cp,
            tc.tile_pool(name="resident", bufs=1) as rp,
            tc.tile_pool(name="dram", bufs=1, space="DRAM") as dp,
        ):
            iota = cp.tile([128, 128], _f32)
            nc.gpsimd.iota(iota[:], pattern=[[1, 128]], base=0,
                           channel_multiplier=0,
                           allow_small_or_imprecise_dtypes=True)
            pidx = cp.tile([128, 1], _f32)
            nc.gpsimd.iota(pidx[:], pattern=[[0, 1]], base=0,
                           channel_multiplier=1,
                           allow_small_or_imprecise_dtypes=True)
            ident = cp.tile([128, 128], _f32)
            nc.vector.tensor_scalar(ident[:], iota[:], pidx[:], None, op0=eq)
            ones_col = cp.tile([128, 1], _f32)
            nc.vector.memset(ones_col[:], 1.0)
            iota_bf = cp.tile([128, 128], _bf16)
            nc.vector.tensor_copy(iota_bf[:], iota[:])
            ones_bf = cp.tile([128, 1], _bf16)
            nc.vector.memset(ones_bf[:], 1.0)
            ones_row = cp.tile([1, 128], _f32)
            nc.vector.memset(ones_row[:], 1.0)
            tiny_g = cp.tile([1, 2], _bf16 if GDT_BF16 else _f32)
            nc.vector.memset(tiny_g[:], 0.0)
            eps_col = cp.tile([128, 1], _f32)
            nc.vector.memset(eps_col[:], float(LN_EPS))
            w_sb = cp.tile([128, 128], _f32)
            nc.sync.dma_start(w_sb[:], w_in)

            # broadcast bias/gamma/beta rows to all 128 partitions via matmul
            bias_bc = cp.tile([128, 128], _f32)
            gamma_bc = cp.tile([128, 128], _f32)
            beta_bc = cp.tile([128, 128], _f32)
            with tc.tile_pool(name="bc", bufs=1) as bcp, \
                 tc.tile_pool(name="bcps", bufs=1, space="PSUM") as bcps:
                for src_ap, dst in ((bias_in, bias_bc), (gamma_in, gamma_bc),
                                    (beta_in, beta_bc)):
                    r = bcp.tile([1, 128], _f32, tag="bcrow")
                    nc.sync.dma_start(r[:], src_ap)
                    ps = bcps.tile([128, 128], _f32, tag="bcps")
                    nc.tensor.matmul(ps[:], lhsT=ones_row[:], rhs=r[:],
                                     start=True, stop=True)
                    nc.scalar.copy(dst[:], ps[:])

            ewt = rp.tile([128, T], _f32)
            nc.sync.dma_start(ewt[:], ewt_in)
            doft = rp.tile([128, T], _f32)
            nc.sync.dma_start(doft[:], doft_in)

            idx_resident = T <= 2560
            if idx_resident:
                idx_sb = rp.tile([128, 8 * T], _i16)
                nc.sync.dma_start(idx_sb[:], idx_in)

            h_sb = rp.tile([128, NB, 128], _f32)    # node features after x@W
            hp_sb = rp.tile([128, NB, 128], gdt)    # dis-scaled, gather dtype
            acc_sb = rp.tile([128, NB, 128], _f32)  # window-0 partial sums
            dis_sb = rp.tile([128, NB], _f32)

            a2a_in1 = dp.tile([C * HALF, D], gdt)
            a2a_in2 = dp.tile([C * HALF, D], gdt)
            hfullA = dp.tile([WROWS, D], gdt)
            hfullB = dp.tile([WROWS, D], gdt)

            def _phases():
                # ---------------- Phase 1: h = x @ W ----------------
                with tc.tile_pool(name="p1", bufs=3) as p1, \
                     tc.tile_pool(name="p1ps", bufs=4, space="PSUM") as p1ps:
                    if "p1" in SKIP:
                        nc.vector.memset(h_sb[:, 0, :1], 0.0)
                    for nb in range(NB):
                        if "p1" in SKIP:
                            break
                        xt = p1.tile([128, 128], _f32, tag="xt")
                        nc.sync.dma_start(xt[:], x_in[nb * 128:(nb + 1) * 128, :])
                        xTp = p1ps.tile([128, 128], _f32, tag="xT")
                        nc.tensor.transpose(xTp[:], xt[:], ident[:])
                        xTs = p1.tile([128, 128], _f32, tag="xTs")
                        nc.scalar.copy(xTs[:], xTp[:])
                        hp = p1ps.tile([128, 128], _f32, tag="hps")
                        nc.tensor.matmul(hp[:], lhsT=xTs[:], rhs=w_sb[:],
                                         start=True, stop=True)
                        nc.scalar.copy(h_sb[:, nb, :], hp[:])

                # ---------------- Pass A: deg ----------------
                with tc.tile_pool(name="pa", bufs=4) as pa, \
                     tc.tile_pool(name="paps", bufs=1, space="PSUM") as paps:
                    degp = paps.tile([128, NB], _f32)
                    if "sa" in SKIP:
                        nc.vector.memset(degp[:, :1], 1.0)
                    def dis_and_exchange(h):
                        # dis + h' scale + replicate + A2A for node-half h
                        lo, hi = h * (NB // 2), (h + 1) * (NB // 2)
                        sl = slice(lo, hi)
                        deg_sb = pa.tile([128, NB // 2], _f32, tag="dg")
                        nc.scalar.copy(deg_sb[:], degp[:, sl])
                        msk = pa.tile([128, NB // 2], _f32, tag="dm")
                        nc.vector.tensor_scalar(msk[:], deg_sb[:], 0.0, None,
                                                op0=mybir.AluOpType.is_gt)
                        dcl = pa.tile([128, NB // 2], _f32, tag="dc")
                        nc.vector.tensor_scalar(dcl[:], deg_sb[:], 1e-30, None,
                                                op0=mybir.AluOpType.max)
                        dsq = pa.tile([128, NB // 2], _f32, tag="ds")
                        nc.scalar.activation(dsq[:], dcl[:], AF.Sqrt)
                        drc = pa.tile([128, NB // 2], _f32, tag="dr")
                        nc.vector.reciprocal(drc[:], dsq[:])
                        nc.vector.tensor_tensor(dis_sb[:, sl], drc[:], msk[:],
                                                op=mul)
                        for nb in range(lo, hi):
                            nc.vector.tensor_scalar(hp_sb[:, nb, :],
                                                    h_sb[:, nb, :],
                                                    dis_sb[:, nb:nb + 1],
                                                    None, op0=mul)
                        a2a = a2a_in1 if h == 0 else a2a_in2
                        hf = hfullA if h == 0 else hfullB
                        for rr in range(C):
                            dst = a2a[rr * HALF:(rr + 1) * HALF, :].rearrange(
                                "(t p) d -> p t d", p=128)
                            nc.sync.dma_start(dst, hp_sb[:, lo:hi, :])
                        if sim_single_core:
                            nc.sync.dma_start(hf[:], a2a[:])
                        else:
                            nc.gpsimd.collective_compute(
                                "AllToAll", mybir.AluOpType.bypass,
                                replica_groups=[list(range(C))],
                                ins=[a2a.opt()], outs=[hf.opt()])

                    for b in range(NB):
                        if "sa" in SKIP:
                            nc.vector.memset(degp[:, :1], 1.0)
                            dis_and_exchange(0)
                            dis_and_exchange(1)
                            break
                        btiles = []
                        for w in (0, 1):
                            if runs[b][w] is not None:
                                btiles.extend(range(runs[b][w][0], runs[b][w][1] + 1))
                        for i, t in enumerate(btiles):
                            S = pa.tile([128, 128], _bf16, tag="SA")
                            nc.vector.tensor_scalar(S[:], iota_bf[:],
                                                    doft[:, t:t + 1],
                                                    ewt[:, t:t + 1],
                                                    op0=eq, op1=mul)
                            nc.tensor.matmul(degp[:, b:b + 1], lhsT=S[:],
                                             rhs=ones_bf[:], start=(i == 0),
                                             stop=(i == len(btiles) - 1))
                        if b == NB // 2 - 1:
                            dis_and_exchange(0)
                        elif b == NB - 1:
                            dis_and_exchange(1)

                # ---------------- Pass B: gather + scatter matmuls ----------------
                with tc.tile_pool(name="pb", bufs=8) as pb, \
                     tc.tile_pool(name="gb", bufs=gbufs) as gbp, \
                     tc.tile_pool(name="pbps", bufs=4, space="PSUM") as pbps, \
                     tc.tile_pool(name="ep", bufs=2) as ep:

                    def epilogue(b, ps):
                        if "epi" in SKIP:
                            return
                        has0 = runs[b][0] is not None
                        if has0 and ps is not None:
                            t2 = ep.tile([128, 128], _f32, tag="e_t2")
                            nc.vector.tensor_tensor(t2[:], ps[:], acc_sb[:, b, :], op=add)
                            t2 = t2[:]
                        elif ps is not None:
                            t2 = ep.tile([128, 128], _f32, tag="e_t2")
                            nc.scalar.copy(t2[:], ps[:])
                            t2 = t2[:]
                        else:
                            t2 = acc_sb[:, b, :]
                        t4 = ep.tile([128, 128], _f32, tag="e_t4")
                        nc.vector.scalar_tensor_tensor(t4[:], t2,
                                                       dis_sb[:, b:b + 1],
                                                       bias_bc[:], op0=mul, op1=add)
                        nsum = ep.tile([128, 1], _f32, tag="e_ns")
                        nc.vector.tensor_reduce(nsum[:], t4[:],
                                                axis=mybir.AxisListType.X,
                                                op=add, negate=True)
                        nmean = ep.tile([128, 1], _f32, tag="e_nm")
                        nc.scalar.mul(nmean[:], nsum[:], 1.0 / 128.0)
                        t5 = ep.tile([128, 128], _f32, tag="e_t5")
                        nc.scalar.activation(t5[:], t4[:], AF.Identity,
                                             bias=nmean[:], scale=1.0)
                        sq = ep.tile([128, 128], _f32, tag="e_sq")
                        vsum = ep.tile([128, 1], _f32, tag="e_vs")
                        nc.scalar.activation(sq[:], t5[:], AF.Square,
                                             accum_out=vsum[:])
                        sd = ep.tile([128, 1], _f32, tag="e_sd")
                        nc.scalar.activation(sd[:], vsum[:], AF.Sqrt,
                                             scale=1.0 / 128.0, bias=eps_col[:])
                        rstd = ep.tile([128, 1], _f32, tag="e_rs")
                        nc.vector.reciprocal(rstd[:], sd[:])
                        t6 = ep.tile([128, 128], _f32, tag="e_t6")
                        nc.vector.scalar_tensor_tensor(t6[:], t5[:], rstd[:],
                                                       gamma_bc[:], op0=mul, op1=mul)
                        nc.vector.tensor_tensor(t6[:], t6[:], beta_bc[:], op=add)
                        osb = ep.tile([128, 128], _f32, tag="e_o")
                        nc.vector.scalar_tensor_tensor(osb[:], t6[:],
                                                       float(LEAKY_ALPHA), t6[:],
                                                       op0=mul,
                                                       op1=mybir.AluOpType.max)
                        nc.sync.dma_start(out_d[b * 128:(b + 1) * 128, :], osb[:])

                    gcall = [0]
                    for w in (0, 1):
                        win_ap = (hfullA if w == 0 else hfullB)[:]
                        nt_left = Tw[w]
                        t0 = t0w[w]
                        cur_ps = None
                        cur_b = -1
                        while nt_left > 0:
                            if nt_left > ch:
                                nt = ch
                            elif w == 1 and nt_left > 24:
                                nt = nt_left - 16
                            else:
                                nt = nt_left
                            gbuf = gbp.tile([128, ch, 128], gdt, tag="g")
                            if idx_resident:
                                idx_ap = idx_sb[:, 8 * t0:8 * (t0 + nt)]
                            else:
                                idx_ch = gbp.tile([128, 8 * ch], _i16, tag="ix")
                                nc.sync.dma_start(idx_ch[:, :8 * nt],
                                                  idx_in[:, 8 * t0:8 * (t0 + nt)])
                                idx_ap = idx_ch[:, :8 * nt]
                            if "g" in SKIP:
                                nc.vector.memset(gbuf[:, 0, :1], 0.0)
                            if "g" not in SKIP:
                                nc.gpsimd.dma_gather(
                                    out_ap=gbuf[:, :nt, :], in_ap=win_ap,
                                    idxs_ap=idx_ap,
                                    num_idxs=128 * nt, num_idxs_reg=128 * nt,
                                    elem_size=128, single_packet=False,
                                    queue_num=gcall[0] % NQ)
                                gcall[0] += 1
                            for s_i in range(nt):
                                t = t0 + s_i
                                b = tiles[t][1]
                                first = runs[b][w][0] == t
                                last = runs[b][w][1] == t
                                if first:
                                    cur_ps = pbps.tile([128, 128], _f32, tag="blk")
                                    cur_b = b
                                    if "mm" in SKIP:
                                        nc.vector.memset(cur_ps[:, :1], 0.0)
                                S = pb.tile([128, 128], gdt, tag="S")
                                if "sb" not in SKIP:
                                    nc.vector.tensor_scalar(S[:], iota_bf[:],
                                                            doft[:, t:t + 1],
                                                            ewt[:, t:t + 1],
                                                            op0=eq, op1=mul)
                                if "mm" not in SKIP:
                                    nc.tensor.matmul(cur_ps[:], lhsT=S[:],
                                                     rhs=gbuf[:, s_i, :],
                                                     start=first, stop=last)
                                if last:
                                    if w == 0:
                                        nc.scalar.copy(acc_sb[:, b, :], cur_ps[:])
                                    else:
                                        epilogue(b, cur_ps)
                                    cur_ps = None
                            t0 += nt
                            nt_left -= nt
                    # blocks with no window-1 tiles: epilogue from acc only
                    for b in range(NB):
                        if runs[b][1] is None:
                            epilogue(b, None)


            if reps == 1:
                _phases()
            else:
                with tc.For_i(0, reps, 1):
                    _phases()

    nc.compile()
    return nc


_CACHE = {}


def _get_compiled(edge_index, edge_weight):
    P, T, ew_t, dof_t, idx16 = _preprocess(edge_index, edge_weight)
    key = P.tobytes()
    if key not in _CACHE:
        tiles, t0w, Tw = _schedule(P)
        _CACHE[key] = _build(P, T, tiles, t0w, Tw)
    return _CACHE[key], ew_t, dof_t, idx16


def kernel(x, edge_index, edge_weight, weight, adaptive_weight, bias,
           ln_gamma, ln_beta):
    x = np.asarray(x, dtype=np.float32)
    weight = np.asarray(weight, dtype=np.float32)
    bias = np.asarray(bias, dtype=np.float32).reshape(1, D)
    gamma = np.asarray(ln_gamma, dtype=np.float32).reshape(1, D)
    beta = np.asarray(ln_beta, dtype=np.float32).reshape(1, D)

    nc, ew_t, dof_t, idx16 = _get_compiled(edge_index, edge_weight)

    in_maps = []
    for c in range(C):
        xp = np.zeros((NPCP, D), np.float32)
        xp[:NPC] = x[c * NPC:(c + 1) * NPC]
        in_maps.append({
            "x": xp, "w": weight, "bias": bias, "gamma": gamma, "beta": beta,
            "ewt": np.ascontiguousarray(ew_t[c]),
            "doft": np.ascontiguousarray(dof_t[c]),
            "idx16": np.ascontiguousarray(idx16[c]),
        })

    res = run_bass_kernel_spmd(nc, in_maps, core_ids=list(range(C)))
    out = np.empty((N, D), np.float32)
    for c in range(C):
        out[c * NPC:(c + 1) * NPC] = res.results[c]["out"][:NPC]
    return out

